# revision 1
# baseline (speedup 1.0000x reference)
"""MoE gate (router) kernel for Trainium2, 8 NeuronCores, data-parallel.

reference: logits = x @ W_g  ([16384,2048] @ [2048,64]); scores = softmax(logits);
           return top-6 (indices, scores).

Strategy
--------
Data-parallel over tokens: each of the 8 cores handles 2048 tokens. The
contraction dim K=2048 lives on SBUF partitions (host pre-arranges). The
kernel is HBM-bandwidth bound, so x is shipped as 3 bytes/element:

    x  = xh + 2^-11 * xl_s          xh fp16,  xl_s = e4m3((x - xh) * 2^11)
    W  is shipped as [Wh16 | Wl_s] fp16 with Wl_s = fp16((W - Wh) * 2^18)
    plus W8 = e4m3(W * 2^7) for the xl product. Both low-order products
    then land in PSUM at the SAME scale 2^18:

    logits = xh@Wh + 2^-18*( xh@Wl_s + xl_s@W8 )

so the fp8 group accumulates directly into psA[:,64:] - the PE performs
the low-order fold, and an engine op never needs two PSUM operands
(NCC_IBVF027 forbids that).

Error budget: the e4m3 quantization of xl dominates at ~5e-6 absolute on
logits (scores' own scale is ~0.1). The min 6th/7th score gap on this
input is 1.7e-6, so a handful of near-tie rows may swap two adjacent,
numerically-equal scores - same accuracy class as the fp16-hi/lo version.

Per 128-token block:
  - one PSUM accumulation group of 32 matmuls into psA [128,128]:
    16x xh_c @ [Wh|Wl_s]_c (fp16, N=128), then 16x xl_c @ W8_c (fp8,
    N=64) accumulating into psA[:,64:]
  - fold: tlo = ACT copy(psA[:,64:] * 2^-18) -> SBUF; lg = DVE add
  - exp with fused accumulate gives the softmax denominator in one ACT op;
    top-8 value/index via DVE max/max_index on raw logits (no max
    subtraction needed, |logits| < ~6); max_index writes the [.,8] index
    staging tile directly; scores = exp(v6) * (1/sum) with the final
    scale on the ACT engine (per-partition scale operand).

DMA design (all measured on this part; the stream is the roofline):
  - Each DMA splits into one descriptor per partition, and each of the
    16 queues moves a flat ~26.7 GB/s for any descriptor >= 4KiB, so the
    practical ceiling is ~425 GB/s/core. One whole-block DMA (6KiB/
    partition) per 128-token block reaches it; larger groups, extra
    rings, or split descriptors all measure slower.
  - ALL x DMAs ride the sync ring (x0 leads it): the scalar sequencer
    runs the per-block activations, and any dma_start queued behind
    them gates the input stream on the epilogue chain. Both W planes
    ride the scalar ring ahead of every activation - scalar is idle
    until block 0's epilogue, and keeping W off the sync ring lets the
    first matmul start as soon as Whl and x0 land (~9.5us).
  - Outputs stage in SBUF and leave as 2 contiguous DMAs at the end;
    idx ships the full top-8 because a [:, :6] strided source shatters
    into 24-byte descriptors (~10us of tail).
"""

import os
import sys

import numpy as np
import ml_dtypes

for _p in ("/opt/trn_rl_repo", "/root/.axon_site/_ro/trn_rl_repo"):
    if os.path.isdir(_p) and _p not in sys.path:
        sys.path.insert(0, _p)

import concourse.bass as bass
import concourse.mybir as mybir
from concourse import bacc, bass_utils
from concourse.tile import TileContext

N_CORES = 8
T_FULL = 16384
K = 2048
E = 64
TOPK = 6
P = 128
KC = K // P  # 16 contraction chunks
LO_SCALE = 2048.0  # 2^11
W8_SCALE = 128.0  # 2^7: e4m3(W * 2^7); xl_s@W8 scale = 2^11 * 2^7 = 2^18
WL_SCALE = 262144.0  # 2^18: fp16((W - Wh) * 2^18) so xh@Wl_s matches
LO_DESCALE = 1.0 / WL_SCALE

_NC_CACHE: dict[int, "bass.Bass"] = {}
LAST_RESULT = None  # BassKernelResults of the most recent kernel() call


def build_nc(t_shard: int = T_FULL // N_CORES) -> "bass.Bass":
    f16 = mybir.dt.float16
    f8 = mybir.dt.float8e4
    f32 = mybir.dt.float32
    i32 = mybir.dt.int32
    u32 = mybir.dt.uint32
    EXP = mybir.ActivationFunctionType.Exp
    COPY = mybir.ActivationFunctionType.Copy

    assert t_shard % P == 0
    nt = t_shard // P  # number of 128-token blocks
    # per (partition, block): 2048 fp16 xh + 2048 fp8 xl packed as 1024
    # "fp16" words -> 3072 fp16 elements, ONE 6KiB contiguous segment.
    SB = 3 * KC * P // 2
    XL0 = KC * P  # fp16-element offset of the packed-fp8 region

    nc = bacc.Bacc()
    x_d = nc.dram_tensor("xp", [P, nt * SB], f16, kind="ExternalInput")
    # W packed partition-major: [K, 2E] -> [p, c, 2E]; one contiguous
    # segment per partition.
    Whl_d = nc.dram_tensor("Whl", [P, KC * 2 * E], f16, kind="ExternalInput")
    W8_d = nc.dram_tensor("W8", [P, KC * E], f8, kind="ExternalInput")
    # outputs in partition-major layout; host reorders. idx carries the
    # full top-8 so the DMA source is contiguous (a [:, :, :6] slice
    # would shatter into 24-byte descriptors); host keeps the first 6.
    idx_o = nc.dram_tensor("idx", [P, nt, 8], i32, kind="ExternalOutput")
    val_o = nc.dram_tensor("val", [P, nt, TOPK], f32, kind="ExternalOutput")

    # DMA shape copied from the measured-best configuration: ONE DMA per
    # 128-token block carrying both planes as a single 6KiB contiguous
    # segment per partition, blocks alternating between the two HWDGE
    # rings (sync, scalar), issue paced by a deep tile pool. Splitting
    # planes onto separate rings, grouping blocks into bigger DMAs, or
    # issuing everything upfront all measurably de-pipeline the stream
    # (DMA execution then alternates with PE instead of overlapping).
    with TileContext(nc) as tc:
        with (
            tc.tile_pool(name="singles", bufs=1) as singles,
            tc.tile_pool(name="xpool", bufs=16) as xpool,
            tc.tile_pool(name="small", bufs=4) as small,
            tc.tile_pool(name="psum", bufs=4, space="PSUM") as psum_pool,
        ):
            # Both W planes ride the scalar ring (idle until block 0's
            # epilogue) so the sync ring leads directly with x0 - the
            # first matmul needs only Whl and x0's first chunks.
            W_sb = singles.tile([P, KC, 2 * E], f16)
            nc.scalar.dma_start(
                out=W_sb, in_=Whl_d[:].rearrange("p (c e) -> p c e", c=KC)
            )
            W8_sb = singles.tile([P, KC, E], f8)
            nc.scalar.dma_start(
                out=W8_sb, in_=W8_d[:].rearrange("p (c e) -> p c e", c=KC)
            )
            # output staging: DMAing outputs per block would make the
            # sync/scalar sequencers (which also feed the input DMAs)
            # block on each block's epilogue - the input stream then
            # lock-steps with compute. Stage in SBUF, 2 DMAs at the end.
            # istage8 is [.., 8] so max_index can write it directly.
            vstage = singles.tile([P, nt, TOPK], f32)
            istage8 = singles.tile([P, nt, 8], u32)

            for b in range(nt):
                xb = xpool.tile([P, SB], f16, tag="xb")
                # all x DMAs on the sync ring with 6KiB descriptors: the
                # DMA queues run at a flat ~26.7GB/s each, and a single
                # ring of whole-block DMAs keeps every queue saturated.
                # 2-block DMAs (12KiB descriptors) measure ~4us slower;
                # the scalar ring carries only W + the activations.
                nc.sync.dma_start(out=xb, in_=x_d[:, b * SB : (b + 1) * SB])
                psA = psum_pool.tile([P, 2 * E], f32, tag="psA")
                # single accumulation group: A0 (start=True) zeroes the
                # whole [128,128] region, the fp8 group then accumulates
                # into psA[:,64:] at the same 2^18 scale.
                for c in range(KC):
                    nc.tensor.matmul(
                        psA,
                        xb[:, c * P : (c + 1) * P],
                        W_sb[:, c],
                        start=(c == 0),
                        stop=False,
                    )
                for c in range(KC):
                    nc.tensor.matmul(
                        psA[:, E:],
                        xb[:, XL0 + c * E : XL0 + (c + 1) * E].bitcast(f8),
                        W8_sb[:, c],
                        start=False,
                        stop=(c == KC - 1),
                    )
                # fold: lg = psA[:,:64] + 2^-18*psA[:,64:] (the PSUM reads
                # must go through ACT/DVE; gpsimd has no PSUM access)
                tlo = small.tile([P, E], f32, tag="tlo")
                nc.scalar.activation(tlo, psA[:, E:], COPY, scale=LO_DESCALE)
                lg = small.tile([P, E], f32, tag="lg")
                nc.vector.tensor_add(lg, psA[:, :E], tlo)
                # softmax + top-6 (no max subtraction; |logits| < ~6)
                erow = small.tile([P, E], f32, tag="erow")
                sume = small.tile([P, 1], f32, tag="sume")
                nc.scalar.activation(erow, lg, EXP, accum_out=sume)
                v8 = small.tile([P, 8], f32, tag="v8")
                nc.vector.max(out=v8, in_=lg)
                nc.vector.max_index(out=istage8[:, b], in_max=v8, in_values=lg)
                rec = small.tile([P, 1], f32, tag="rec")
                nc.vector.reciprocal(rec, sume)
                ev = small.tile([P, TOPK], f32, tag="ev")
                nc.scalar.activation(ev, v8[:, :TOPK], EXP)
                nc.scalar.activation(vstage[:, b], ev, COPY, scale=rec)

            nc.sync.dma_start(out=val_o[:], in_=vstage)
            nc.scalar.dma_start(out=idx_o[:], in_=istage8[:].bitcast(i32))
    if not nc.is_finalized():
        nc.finalize()
    return nc


def _get_nc(t_shard: int) -> "bass.Bass":
    if t_shard not in _NC_CACHE:
        _NC_CACHE[t_shard] = build_nc(t_shard)
    return _NC_CACHE[t_shard]


def pack_core_input(xh: np.ndarray, xl8: np.ndarray) -> np.ndarray:
    """[t_shard, K] fp16 + fp8 -> [P, nt*3072] 'fp16': per (partition,
    block) 2048 fp16 xh then 2048 fp8 xl packed as 1024 fp16 words."""
    t_shard = xh.shape[0]
    nt = t_shard // P
    SB = 3 * KC * P // 2
    out = np.empty((P, nt, SB), np.float16)
    hiT = xh.T.reshape(KC, P, nt, P).transpose(1, 2, 0, 3)  # [p, b, c, t]
    out[:, :, : KC * P] = hiT.reshape(P, nt, KC * P)
    loT = (
        xl8.view(np.uint8).T.reshape(KC, P, nt, P).transpose(1, 2, 0, 3)
    )  # [p, b, c, t] bytes
    out[:, :, KC * P :] = (
        np.ascontiguousarray(loT.reshape(P, nt, KC * P))
        .view(np.uint16)
        .view(np.float16)
    )
    return np.ascontiguousarray(out.reshape(P, nt * SB))


def kernel(x: np.ndarray, W_g: np.ndarray, **run_kwargs):
    global LAST_RESULT
    x = np.asarray(x, dtype=np.float32)
    W = np.asarray(W_g, dtype=np.float32)
    t_shard = x.shape[0] // N_CORES
    nc = _get_nc(t_shard)

    xh = x.astype(np.float16)
    xl8 = ((x - xh.astype(np.float32)) * np.float32(LO_SCALE)).astype(
        ml_dtypes.float8_e4m3
    )
    Wh = W.astype(np.float16)
    Wl = ((W - Wh.astype(np.float32)) * np.float32(WL_SCALE)).astype(np.float16)
    Wstk = np.concatenate([Wh, Wl], axis=1)  # [K, 2E] fp16
    Whl = np.ascontiguousarray(
        Wstk.reshape(KC, P, 2 * E).transpose(1, 0, 2).reshape(P, KC * 2 * E)
    )
    W8 = (W * np.float32(W8_SCALE)).astype(ml_dtypes.float8_e4m3)
    W8p = np.ascontiguousarray(
        W8.reshape(KC, P, E).transpose(1, 0, 2).reshape(P, KC * E)
    )
    in_maps = [
        {
            "xp": pack_core_input(
                xh[c * t_shard : (c + 1) * t_shard],
                xl8[c * t_shard : (c + 1) * t_shard],
            ),
            "Whl": Whl,
            "W8": W8p,
        }
        for c in range(N_CORES)
    ]
    res = bass_utils.run_bass_kernel_spmd(
        nc, in_maps, core_ids=list(range(N_CORES)), **run_kwargs
    )
    LAST_RESULT = res
    # device layout is [P, nt, 8/TOPK]; token t = tile*P + p -> [t_shard, .]
    idx = np.concatenate(
        [
            np.moveaxis(r["idx"], 0, 1).reshape(t_shard, 8)[:, :TOPK]
            for r in res.results
        ],
        axis=0,
    ).astype(np.int32)
    val = np.concatenate(
        [np.moveaxis(r["val"], 0, 1).reshape(t_shard, TOPK) for r in res.results],
        axis=0,
    ).astype(np.float32)
    return idx, val



# revision 3
# speedup vs baseline: 1.2535x; 1.2535x over previous
"""MoE gate (router) kernel for Trainium2, 8 NeuronCores, data-parallel.

reference: logits = x @ W_g  ([16384,2048] @ [2048,64]); scores = softmax(logits);
           return top-6 (indices, scores).

Strategy (v2: fp16 stream)
--------------------------
Data-parallel over tokens: each of the 8 cores handles 2048 tokens. The
contraction dim K=2048 lives on SBUF partitions (host pre-arranges). The
kernel is HBM-bandwidth bound; v2 ships x as 2 bytes/element:

    x  ~ fp16(x);   W is shipped as [Wh | Wl_s] fp16 with
    Wl_s = fp16((W - Wh) * 2^11)   (scaled into fp16 normal range)

    logits = xh@Wh + 2^-11 * (xh@Wl_s)

W's quantization error is thereby removed; the remaining error is x's
fp16 rounding: ~2.2e-4 max relative on scores, and ~71/16384 rows swap
two adjacent, numerically-equal scores (min 6th/7th gap on this input is
1.7e-6).  Measured off-line: rv_idx ~ 6e-4, rv_val ~ 3e-8 - both far
under the 2e-2 gate, vs ~30us of DMA saved over the 3-byte hi+lo scheme.

Per 128-token block:
  - one PSUM accumulation group of 16 fp16 matmuls into psA [128,128]
    (N=128 carries both W planes per chunk)
  - fold: tlo = ACT copy(psA[:,64:] * 2^-11) -> SBUF; lg = DVE add
  - erow = exp(lg) with fused accumulate -> softmax denominator in one
    ACT op (no max subtraction needed, |logits| < ~6)
  - top-8 value/index via DVE max/max_index on erow (exp is monotone so
    indices match lg's); max_index writes the [.,8] index staging tile
  - rec = 1/sum via reciprocal_approx_fast (custom DVE op, ~18 bits,
    avoids the table-based InstReciprocal -> no DVE-table reload DMAs
    stealing queue E64 mid-stream)
  - scores = v8[:, :6] * rec on DVE (tensor_scalar_mul, per-partition
    scalar) - keeps the scalar engine at ~1.0us/block, under the
    1.23us/block DMA cadence.

Engine budget per block (measured op costs): DMA 1.23us (the roofline),
scalar 1.02us (fold-copy 390 + exp 347 + accum-read 278), DVE ~1.0us,
PE ~0.9us (16 fp16 matmuls at 56ns cadence).

DMA design (measured on this part):
  - Each DMA splits into one descriptor per partition; each of the 16
    queue engines moves a flat ~26.7 GB/s for descriptors >= 4KiB, so
    the practical ceiling is ~425 GB/s/core. One whole-block DMA (now
    exactly 4KiB/partition) per 128-token block rides it.
  - ALL x DMAs ride the sync ring (x0 leads it); Whl rides the scalar
    ring ahead of every activation.
  - Outputs stage in SBUF and leave as 2 contiguous DMAs at the end;
    idx ships the full top-8 because a [:, :6] strided source shatters
    into 24-byte descriptors (~10us of tail).
"""

import os
import sys

import numpy as np

for _p in ("/opt/trn_rl_repo", "/root/.axon_site/_ro/trn_rl_repo"):
    if os.path.isdir(_p) and _p not in sys.path:
        sys.path.insert(0, _p)

import concourse.bass as bass
import concourse.mybir as mybir
from concourse import bacc, bass_utils
from concourse.tile import TileContext

N_CORES = 8
T_FULL = 16384
K = 2048
E = 64
TOPK = 6
P = 128
KC = K // P  # 16 contraction chunks
WL_SCALE = 2048.0  # 2^11: fp16((W - Wh) * 2^11) keeps Wl in normal range
LO_DESCALE = 1.0 / WL_SCALE

_NC_CACHE: dict[int, "bass.Bass"] = {}
LAST_RESULT = None  # BassKernelResults of the most recent kernel() call


def build_nc(t_shard: int = T_FULL // N_CORES) -> "bass.Bass":
    f16 = mybir.dt.float16
    f32 = mybir.dt.float32
    i32 = mybir.dt.int32
    u32 = mybir.dt.uint32
    EXP = mybir.ActivationFunctionType.Exp
    COPY = mybir.ActivationFunctionType.Copy

    assert t_shard % P == 0
    nt = t_shard // P  # number of 128-token blocks
    SB = KC * P  # 2048 fp16 = 4KiB per (partition, block)

    nc = bacc.Bacc()
    x_d = nc.dram_tensor("xp", [P, nt * SB], f16, kind="ExternalInput")
    # W packed partition-major: [K, 2E] -> [p, c, 2E]; one contiguous
    # segment per partition.
    Whl_d = nc.dram_tensor("Whl", [P, KC * 2 * E], f16, kind="ExternalInput")
    # outputs in partition-major layout; host reorders. idx carries the
    # full top-8 so the DMA source is contiguous (a [:, :, :6] slice
    # would shatter into 24-byte descriptors); host keeps the first 6.
    idx_o = nc.dram_tensor("idx", [P, nt, 8], i32, kind="ExternalOutput")
    val_o = nc.dram_tensor("val", [P, nt, TOPK], f32, kind="ExternalOutput")

    with TileContext(nc) as tc:
        with (
            tc.tile_pool(name="singles", bufs=1) as singles,
            tc.tile_pool(name="xpool", bufs=16) as xpool,
            tc.tile_pool(name="small", bufs=4) as small,
            tc.tile_pool(name="psum", bufs=4, space="PSUM") as psum_pool,
        ):
            # W rides the scalar ring (idle until block 0's epilogue) so
            # the sync ring leads directly with x0.
            W_sb = singles.tile([P, KC, 2 * E], f16)
            nc.scalar.dma_start(
                out=W_sb, in_=Whl_d[:].rearrange("p (c e) -> p c e", c=KC)
            )
            # output staging: DMAing outputs per block would make the
            # sync/scalar sequencers (which also feed the input DMAs)
            # block on each block's epilogue - the input stream then
            # lock-steps with compute. Stage in SBUF, 2 DMAs at the end.
            # istage8 is [.., 8] so max_index can write it directly.
            vstage = singles.tile([P, nt, TOPK], f32)
            istage8 = singles.tile([P, nt, 8], u32)

            for b in range(nt):
                xb = xpool.tile([P, SB], f16, tag="xb")
                # all x DMAs on the sync ring with 4KiB descriptors: the
                # DMA queues run at a flat ~26.7GB/s each, and a single
                # ring of whole-block DMAs keeps every queue saturated.
                nc.sync.dma_start(out=xb, in_=x_d[:, b * SB : (b + 1) * SB])
                psA = psum_pool.tile([P, 2 * E], f32, tag="psA")
                # single accumulation group of 16 fp16 matmuls; N=128
                # carries [Wh | Wl_s] per chunk.
                for c in range(KC):
                    nc.tensor.matmul(
                        psA,
                        xb[:, c * P : (c + 1) * P],
                        W_sb[:, c],
                        start=(c == 0),
                        stop=(c == KC - 1),
                    )
                # fold: lg = psA[:,:64] + 2^-11*psA[:,64:] (the PSUM reads
                # must go through ACT/DVE; an engine op never takes two
                # PSUM operands)
                tlo = small.tile([P, E], f32, tag="tlo")
                nc.scalar.activation(tlo, psA[:, E:], COPY, scale=LO_DESCALE)
                lg = small.tile([P, E], f32, tag="lg")
                nc.vector.tensor_add(lg, psA[:, :E], tlo)
                # softmax + top-6 (no max subtraction; |logits| < ~6).
                # max/max_index run on erow = exp(lg): exp is monotone so
                # the indices match, and it saves the exp(v6) ACT op the
                # scalar engine can't afford at the 1.23us DMA cadence.
                erow = small.tile([P, E], f32, tag="erow")
                sume = small.tile([P, 1], f32, tag="sume")
                nc.scalar.activation(erow, lg, EXP, accum_out=sume)
                v8 = small.tile([P, 8], f32, tag="v8")
                nc.vector.max(out=v8, in_=erow)
                nc.vector.max_index(out=istage8[:, b], in_max=v8, in_values=erow)
                rec = small.tile([P, 1], f32, tag="rec")
                nc.vector.reciprocal_approx_fast(out=rec, in_=sume)
                nc.vector.tensor_scalar_mul(vstage[:, b], v8[:, :TOPK], rec)

            nc.sync.dma_start(out=val_o[:], in_=vstage)
            nc.scalar.dma_start(out=idx_o[:], in_=istage8[:].bitcast(i32))
    if not nc.is_finalized():
        nc.finalize()
    return nc


def _get_nc(t_shard: int) -> "bass.Bass":
    if t_shard not in _NC_CACHE:
        _NC_CACHE[t_shard] = build_nc(t_shard)
    return _NC_CACHE[t_shard]


def pack_core_input(xh: np.ndarray) -> np.ndarray:
    """[t_shard, K] fp16 -> [P, nt*2048]: per (partition, block) the 16
    chunk rows of 128 tokens, 4KiB contiguous."""
    t_shard = xh.shape[0]
    nt = t_shard // P
    hiT = xh.T.reshape(KC, P, nt, P).transpose(1, 2, 0, 3)  # [p, b, c, t]
    return np.ascontiguousarray(hiT.reshape(P, nt * KC * P))


def kernel(x: np.ndarray, W_g: np.ndarray, **run_kwargs):
    global LAST_RESULT
    x = np.asarray(x, dtype=np.float32)
    W = np.asarray(W_g, dtype=np.float32)
    t_shard = x.shape[0] // N_CORES
    nc = _get_nc(t_shard)

    xh = x.astype(np.float16)
    Wh = W.astype(np.float16)
    Wl = ((W - Wh.astype(np.float32)) * np.float32(WL_SCALE)).astype(np.float16)
    Wstk = np.concatenate([Wh, Wl], axis=1)  # [K, 2E] fp16
    Whl = np.ascontiguousarray(
        Wstk.reshape(KC, P, 2 * E).transpose(1, 0, 2).reshape(P, KC * 2 * E)
    )
    in_maps = [
        {
            "xp": pack_core_input(xh[c * t_shard : (c + 1) * t_shard]),
            "Whl": Whl,
        }
        for c in range(N_CORES)
    ]
    res = bass_utils.run_bass_kernel_spmd(
        nc, in_maps, core_ids=list(range(N_CORES)), **run_kwargs
    )
    LAST_RESULT = res
    # device layout is [P, nt, 8/TOPK]; token t = tile*P + p -> [t_shard, .]
    idx = np.concatenate(
        [
            np.moveaxis(r["idx"], 0, 1).reshape(t_shard, 8)[:, :TOPK]
            for r in res.results
        ],
        axis=0,
    ).astype(np.int32)
    val = np.concatenate(
        [np.moveaxis(r["val"], 0, 1).reshape(t_shard, TOPK) for r in res.results],
        axis=0,
    ).astype(np.float32)
    return idx, val


# revision 9
# speedup vs baseline: 1.3092x; 1.0444x over previous
"""MoE gate (router) kernel for Trainium2, 8 NeuronCores, data-parallel.

reference: logits = x @ W_g  ([16384,2048] @ [2048,64]); scores = softmax(logits);
           return top-6 (indices, scores).

Strategy (v3: fp16 stream, foldless)
------------------------------------
Data-parallel over tokens: each of the 8 cores handles 2048 tokens. The
contraction dim K=2048 lives on SBUF partitions (host pre-arranges). The
kernel is HBM-bandwidth bound; x and W ship as plain fp16 (2B/elem).

fp16 rounding of x AND W gives ~3.6e-4 max relative error on scores and
~97/16384 rows swap two adjacent, numerically-equal scores (min 6th/7th
score gap on this input is 1.7e-6). Measured off-line: rv_idx ~ 8.8e-4,
rv_val ~ 6e-8 - both far under the 2e-2 gate, vs ~30us of DMA saved
over the 3-byte hi+lo scheme.

v2 carried a Wl correction plane and folded psA[:,:64] + 2^-11*
psA[:,64:] per block; the fold's scalar->DVE->scalar ping-pong put
~450ns of dead time on the in-order scalar queue per hop, making the
epilogue cadence (~1.4us/block) exceed the DMA cadence (1.23us/block) -
the PE then trailed data arrival by ~6us. Dropping Wl (71 -> 97 bad
rows, both noise vs the gate) removes the fold entirely.

Per 128-token block:
  - one PSUM accumulation group of 16 fp16 matmuls into psA [128,64]
  - erow = exp(psA) with fused accumulate reads PSUM directly and gives
    the softmax denominator in one ACT op (no max subtraction needed,
    |logits| < ~6); this is also the only PSUM reader, freeing the bank
  - top-8 value/index via DVE max/max_index on erow (exp is monotone so
    indices match the logits'); max_index writes the [.,8] index staging
  - rec = 1/sum via reciprocal_approx_fast (custom DVE op, ~18 bits,
    faster than table-based InstReciprocal and less DVE-table traffic)
  - scores = v8[:, :6] * rec on DVE (tensor_scalar_mul, per-partition
    scalar)

Engine budget per block (measured op costs): DMA 1.23us (the roofline),
PE 1.07us (16 fp16 matmuls), DVE ~0.88us, scalar ~0.63us.

DMA design (measured on this part):
  - Each DMA splits into one descriptor per partition; each of the 16
    queue engines moves a flat ~26.7 GB/s for descriptors >= 4KiB, so
    the practical ceiling is ~425 GB/s/core. One whole-block DMA (now
    exactly 4KiB/partition) per 128-token block rides it.
  - W leads the SYNC ring, then all x blocks: same-queue FIFO means W's
    packets complete before x0's, so block 0's matmuls never wait on W
    (v2 had W on the scalar ring; the DGE served it after x0-x2,
    stalling the PE ~1.6us at the start).
  - Outputs stage in SBUF and leave as 2 contiguous DMAs at the end;
    idx ships the full top-8 because a [:, :6] strided source shatters
    into 24-byte descriptors (~10us of tail).
"""

import os
import sys

import numpy as np

for _p in ("/opt/trn_rl_repo", "/root/.axon_site/_ro/trn_rl_repo"):
    if os.path.isdir(_p) and _p not in sys.path:
        sys.path.insert(0, _p)

import concourse.bass as bass
import concourse.mybir as mybir
from concourse import bacc, bass_utils
from concourse.tile import TileContext

N_CORES = 8
T_FULL = 16384
K = 2048
E = 64
TOPK = 6
P = 128
KC = K // P  # 16 contraction chunks

_NC_CACHE: dict[int, "bass.Bass"] = {}
LAST_RESULT = None  # BassKernelResults of the most recent kernel() call


def build_nc(t_shard: int = T_FULL // N_CORES) -> "bass.Bass":
    f16 = mybir.dt.float16
    f32 = mybir.dt.float32
    i32 = mybir.dt.int32
    u32 = mybir.dt.uint32
    EXP = mybir.ActivationFunctionType.Exp

    assert t_shard % P == 0
    nt = t_shard // P  # number of 128-token blocks
    SB = KC * P  # 2048 fp16 = 4KiB per (partition, block)

    nc = bacc.Bacc()
    x_d = nc.dram_tensor("xp", [P, nt * SB], f16, kind="ExternalInput")
    # W packed partition-major: [K, E] -> [p, c, E]; one contiguous
    # segment per partition.
    Wh_d = nc.dram_tensor("Wh", [P, KC * E], f16, kind="ExternalInput")
    # outputs in partition-major layout; host reorders. idx carries the
    # full top-8 so the DMA source is contiguous (a [:, :, :6] slice
    # would shatter into 24-byte descriptors); host keeps the first 6.
    idx_o = nc.dram_tensor("idx", [P, nt, 8], i32, kind="ExternalOutput")
    val_o = nc.dram_tensor("val", [P, nt, TOPK], f32, kind="ExternalOutput")

    with TileContext(nc) as tc:
        with (
            tc.tile_pool(name="singles", bufs=1) as singles,
            tc.tile_pool(name="xpool", bufs=16) as xpool,
            tc.tile_pool(name="small", bufs=4) as small,
            tc.tile_pool(name="psum", bufs=4, space="PSUM") as psum_pool,
        ):
            # W leads the sync ring: same-queue FIFO puts its packets
            # ahead of x0's, so block 0's matmuls never wait on W.
            W_sb = singles.tile([P, KC, E], f16)
            nc.sync.dma_start(
                out=W_sb, in_=Wh_d[:].rearrange("p (c e) -> p c e", c=KC)
            )
            # output staging: DMAing outputs per block would make the
            # sync/scalar sequencers (which also feed the input DMAs)
            # block on each block's epilogue - the input stream then
            # lock-steps with compute. Stage in SBUF, 2 DMAs at the end.
            # istage8 is [.., 8] so max_index can write it directly.
            vstage = singles.tile([P, nt, TOPK], f32)
            istage8 = singles.tile([P, nt, 8], u32)

            for b in range(nt):
                xb = xpool.tile([P, SB], f16, tag="xb")
                # all x DMAs on the sync ring with 4KiB descriptors: the
                # DMA queues run at a flat ~26.7GB/s each, and a single
                # ring of whole-block DMAs keeps every queue saturated.
                nc.sync.dma_start(out=xb, in_=x_d[:, b * SB : (b + 1) * SB])
                psA = psum_pool.tile([P, E], f32, tag="psA")
                # single accumulation group of 16 fp16 matmuls (N=64)
                for c in range(KC):
                    nc.tensor.matmul(
                        psA,
                        xb[:, c * P : (c + 1) * P],
                        W_sb[:, c],
                        start=(c == 0),
                        stop=(c == KC - 1),
                    )
                # softmax + top-6 (no max subtraction; |logits| < ~6).
                # EXP reads PSUM directly (sole reader -> frees the bank)
                # and the fused accumulate gives the denominator free.
                # max/max_index run on erow = exp(logits): exp is
                # monotone so the indices match the logits'.
                erow = small.tile([P, E], f32, tag="erow")
                sume = small.tile([P, 1], f32, tag="sume")
                nc.scalar.activation(erow, psA, EXP, accum_out=sume)
                v8 = small.tile([P, 8], f32, tag="v8")
                nc.vector.max(out=v8, in_=erow)
                nc.vector.max_index(out=istage8[:, b], in_max=v8, in_values=erow)
                rec = small.tile([P, 1], f32, tag="rec")
                nc.vector.reciprocal_approx_fast(out=rec, in_=sume)
                nc.vector.tensor_scalar_mul(vstage[:, b], v8[:, :TOPK], rec)

            nc.sync.dma_start(out=val_o[:], in_=vstage)
            nc.scalar.dma_start(out=idx_o[:], in_=istage8[:].bitcast(i32))
    if not nc.is_finalized():
        nc.finalize()
    return nc


def _get_nc(t_shard: int) -> "bass.Bass":
    if t_shard not in _NC_CACHE:
        _NC_CACHE[t_shard] = build_nc(t_shard)
    return _NC_CACHE[t_shard]


def pack_core_input(xh: np.ndarray) -> np.ndarray:
    """[t_shard, K] fp16 -> [P, nt*2048]: per (partition, block) the 16
    chunk rows of 128 tokens, 4KiB contiguous."""
    t_shard = xh.shape[0]
    nt = t_shard // P
    hiT = xh.T.reshape(KC, P, nt, P).transpose(1, 2, 0, 3)  # [p, b, c, t]
    return np.ascontiguousarray(hiT.reshape(P, nt * KC * P))


def kernel(x: np.ndarray, W_g: np.ndarray, **run_kwargs):
    global LAST_RESULT
    x = np.asarray(x, dtype=np.float32)
    W = np.asarray(W_g, dtype=np.float32)
    t_shard = x.shape[0] // N_CORES
    nc = _get_nc(t_shard)

    xh = x.astype(np.float16)
    Wh = W.astype(np.float16)  # [K, E]
    Whp = np.ascontiguousarray(
        Wh.reshape(KC, P, E).transpose(1, 0, 2).reshape(P, KC * E)
    )
    in_maps = [
        {
            "xp": pack_core_input(xh[c * t_shard : (c + 1) * t_shard]),
            "Wh": Whp,
        }
        for c in range(N_CORES)
    ]
    res = bass_utils.run_bass_kernel_spmd(
        nc, in_maps, core_ids=list(range(N_CORES)), **run_kwargs
    )
    LAST_RESULT = res
    # device layout is [P, nt, 8/TOPK]; token t = tile*P + p -> [t_shard, .]
    idx = np.concatenate(
        [
            np.moveaxis(r["idx"], 0, 1).reshape(t_shard, 8)[:, :TOPK]
            for r in res.results
        ],
        axis=0,
    ).astype(np.int32)
    val = np.concatenate(
        [np.moveaxis(r["val"], 0, 1).reshape(t_shard, TOPK) for r in res.results],
        axis=0,
    ).astype(np.float32)
    return idx, val
